# revision 17
# baseline (speedup 1.0000x reference)
"""Trainium2 Bass kernel for NeuralComplexityLoss (sample-entropy MSE).

Contract: kernel(predictions, targets) -> np.float32 scalar (shape ()),
matching reference.reference(). Self-contained: hardcodes shapes/sharding.

Strategy (diagonal layout, quad-batched, engine-balanced)
---------------------------------------------------------
128 signals (64 pred + 64 targ, length T=1024, host-normalized) are sharded
16 per core across 8 NeuronCores. Per signal, sample-entropy match counts
accumulate per diagonal d = j - i:

  B[d, i] = [ |x_i - x_{i+d}| <= R ]
  cnt2 += sum_i B[d,i] B[d,i+1],   cnt3 += sum_i B[d,i] B[d,i+1] B[d,i+2]

Packing: 4 signals x 32 diagonals per 128 SBUF partitions (lane = dd*4+sig).
The 32 diagonal-blocks pair into 8 constant-width quads {q,15-q,16+q,31-q}
(width 2116 incl. 2 pad columns per segment). x is padded with an increasing
ramp (50000+100k) so every out-of-range position yields |S| >> R.

Per (group, quad):
  S    = X0 - Z          fp32 -> fp16   (widest blocks on DVE, rest GPSIMD)
  A    = |S|             ACT Abs -> fp16
  then one of two count flavors (split tuned to balance engines):
   DVE-form:  B = (A<=R) [DVE 4x];  c2 = (A0<=R)*B1 [stt+accum];
              c3 = (A2<=R)*c2 [stt+accum]
   ACT-form:  m2 = max(A0,A1), m3 = max(m2,A2) [DVE tt 2x];
              sign-counts: accum sum(Sign(R-m)) on ACT; host decodes
              cnt = (acc + 32*W)/2  (no exact ties: 0.2 not fp16-exact)

Row sums land in stats [128, 64]; one PE matmul with a (k%4==m) selector
gives per-signal sums [4, 64]. Host subtracts the single per-diagonal
spurious c2 term (O(N), exact fp16 mirror), forms matches_m = 2*(cnt2-spur)+N,
matches_m1 = 2*cnt3+N, entropies, and the final MSE.
"""

import os

import numpy as np

B, C, T = 4, 16, 1024
M = 2
R = 0.2
EPS = 1e-8
N = T - M                      # 1022 templates
NCORES = 8
NSIG = 2 * B * C               # 128 signals
S_PER_CORE = NSIG // NCORES    # 16
PADLEN = 1152
NG = 4                         # signal groups of 4 per core
NQ = 8                         # quads per group
QUADS = [[q, 15 - q, 16 + q, 31 - q] for q in range(NQ)]
QW = sum((1023 - 32 * b) + 2 for b in QUADS[0])  # 2116, same for all quads
N_DVE_FORM = int(os.environ.get("KPHI", "4"))    # DVE-form quad count (4 => parity-alternating)


_DVE_FORM_SET = {(i * NQ) // N_DVE_FORM for i in range(N_DVE_FORM)} if N_DVE_FORM else set()


def _is_dve_form(q):
    return q in _DVE_FORM_SET
N_PE = int(os.environ.get("KPE", "12"))          # widest blocks' S on PE (matmul)

_CACHE = {}
LAST_RESULTS = None


def _split_excess_waits(nc, maxw=1):
    """This walrus codegen accepts only one sync-wait per instruction:
    hoist extras onto preceding single-wait NOPs on the same engine."""
    import bass_rust
    import concourse.mybir as mybir

    n_split = 0
    for bb in nc.main_func.blocks:
        insts = bb.instructions
        i = 0
        while i < len(insts):
            ins = insts[i]
            si = ins.sync_info
            waits = list(si.on_wait) if si is not None and si.on_wait else []
            if len(waits) > maxw:
                extra, keep = waits[:-maxw], waits[-maxw:]
                nops = []
                for j, w in enumerate(extra):
                    nop = bass_rust.InstNoOp(
                        name=f"{ins.name}-wsplit{j}", ins=[], outs=[]
                    )
                    nop.engine = ins.engine
                    nop.sync_info = mybir.SyncInfo(on_wait=[w], on_update=[])
                    nops.append(nop)
                si.on_wait = keep
                insts[i:i] = nops
                i += len(nops)
                n_split += 1
            i += 1
    return n_split


def _build():
    import concourse.bass as bass
    import concourse.tile as tile
    from concourse import mybir
    from concourse.alu_op_type import AluOpType

    f32 = mybir.dt.float32
    f16 = mybir.dt.float16

    nc = bass.Bass(trn_type="TRN2", num_devices=NCORES)
    x = nc.dram_tensor("x", [S_PER_CORE, PADLEN], f32, kind="ExternalInput")
    out = nc.dram_tensor("cnt", [4, 2 * NQ * NG], f32, kind="ExternalOutput")

    sel_np = np.zeros((128, 4), dtype=np.float32)
    sel_np[np.arange(128), np.arange(128) % 4] = 1.0
    sel_dram = nc.inline_tensor(sel_np, name="sel")
    p4_np = np.zeros((128, 128), dtype=np.float32)
    p4_np[np.arange(128) % 4, np.arange(128)] = 1.0
    p4_dram = nc.inline_tensor(p4_np, name="p4sel")
    ni_np = (-np.eye(128, dtype=np.float32))
    ni_dram = nc.inline_tensor(ni_np, name="negi")

    xa = x.ap()
    GB = int(os.environ.get("KGB", "1"))   # groups per batch
    NGB = NG // GB
    CH = 512        # psum chunk columns (one bank of fp32)
    with tile.TileContext(nc) as tc:
        with (
            tc.tile_pool(name="singles", bufs=1) as singles,
            tc.tile_pool(name="wrk", bufs=int(os.environ.get("KBUFS", "9"))) as wrk,
            tc.tile_pool(name="pchunk", bufs=3, space="PSUM") as pchunk,
            tc.tile_pool(name="pred", bufs=1, space="PSUM") as pred,
        ):
            # Z[l, 1026*g + n] = xpad[4g+sig, n + dd],  l = dd*4 + sig
            Zb = singles.tile([128, 4 * 1026], f32)
            X0b = singles.tile([128, 4 * 1025], f32)
            for g in range(NG):
                goff = g * 4 * PADLEN
                nc.sync.dma_start(
                    out=Zb[:, g * 1026 : (g + 1) * 1026],
                    in_=bass.AP(
                        tensor=xa.tensor,
                        offset=xa.offset + goff,
                        ap=[[1, 32], [PADLEN, 4], [1, 1026]],
                    ),
                )
                nc.sync.dma_start(
                    out=X0b[:, g * 1025 : (g + 1) * 1025],
                    in_=bass.AP(
                        tensor=xa.tensor,
                        offset=xa.offset + goff,
                        ap=[[0, 32], [PADLEN, 4], [1, 1025]],
                    ),
                )
            stats = singles.tile([128, 2 * NQ * NG], f32)
            nc.vector.memset(stats, 0.0)
            rbias = singles.tile([128, 1], f32)
            nc.vector.memset(rbias, float(R))
            selt = singles.tile([128, 4], f32)
            nc.sync.dma_start(out=selt, in_=sel_dram[:, :])
            p4t = singles.tile([128, 128], f32)
            nc.sync.dma_start(out=p4t, in_=p4_dram[:, :])
            nit = singles.tile([128, 128], f32)
            nc.sync.dma_start(out=nit, in_=ni_dram[:, :])

            za = Zb[:, :]
            xo = X0b[:, :]

            def gview(base_ap, pitch, g0, off, w):
                return bass.AP(
                    tensor=base_ap.tensor,
                    offset=base_ap.offset + g0 * pitch + off,
                    ap=[base_ap.ap[0], [pitch, GB], [1, w]],
                )

            for gb in range(NGB):
                g0 = gb * GB
                for q in range(NQ):
                    dve_form = _is_dve_form(q)
                    peb = [b for b in QUADS[q] if b < N_PE]
                    dvb = [b for b in QUADS[q] if b >= N_PE]
                    layout = peb + dvb
                    offs = {}
                    off = 0
                    for b in layout:
                        offs[b] = off
                        off += (1023 - 32 * b) + 2
                    dve_off = offs[dvb[0]] if dvb else QW

                    S = wrk.tile([128, GB * QW], f16)
                    sa = S[:, :]
                    A = wrk.tile([128, GB * QW], f16)
                    aa = A[:, :]

                    # PE blocks: S = x_i - x_{i+d} via two matmuls into PSUM,
                    # then ACT Abs(psum) -> A segment (fp16) directly.
                    # 2-bank psum tiles: two 512-col matmul pairs share one
                    # Abs op; same-weight matmuls grouped to halve LdWeights.
                    for b in peb:
                        WB = 1023 - 32 * b
                        d0 = 1 + 32 * b
                        for gg in range(GB):
                            g = g0 + gg
                            c0 = 0
                            while c0 < WB + 2:
                                wc = min(2 * CH, WB + 2 - c0)
                                ps = pchunk.tile([128, 2 * CH], f32)
                                halves = []
                                h0 = 0
                                while h0 < wc:
                                    hw = min(CH, wc - h0)
                                    halves.append((h0, hw))
                                    h0 += hw
                                for h0, hw in halves:
                                    nc.tensor.matmul(
                                        ps[:, h0 : h0 + hw], p4t,
                                        Zb[:, g * 1026 + c0 + h0 :
                                           g * 1026 + c0 + h0 + hw],
                                        start=True, stop=False,
                                        skip_group_check=True,
                                    )
                                for h0, hw in halves:
                                    nc.tensor.matmul(
                                        ps[:, h0 : h0 + hw], nit,
                                        Zb[:, g * 1026 + d0 + c0 + h0 :
                                           g * 1026 + d0 + c0 + h0 + hw],
                                        start=False, stop=True,
                                        skip_group_check=True,
                                    )
                                ab = gg * QW + offs[b] + c0
                                nc.scalar.activation(
                                    out=A[:, ab : ab + wc], in_=ps[:, 0:wc],
                                    func=mybir.ActivationFunctionType.Abs,
                                )
                                c0 += wc
                    # DVE blocks: S via tensor_tensor, then one Abs per region
                    for b in dvb:
                        WB = 1023 - 32 * b
                        d0 = 1 + 32 * b
                        nc.vector.tensor_tensor(
                            out=gview(sa, QW, 0, offs[b], WB + 2),
                            in0=gview(xo, 1025, g0, 0, WB + 2),
                            in1=gview(za, 1026, g0, d0, WB + 2),
                            op=AluOpType.subtract,
                        )
                    if dvb:
                        nc.scalar.activation(
                            out=gview(aa, QW, 0, dve_off, QW - dve_off),
                            in_=gview(sa, QW, 0, dve_off, QW - dve_off),
                            func=mybir.ActivationFunctionType.Abs,
                        )

                    u1 = wrk.tile([128, GB * QW], f16)
                    u2 = wrk.tile([128, GB * QW], f16)
                    if dve_form:
                        Bt = u1
                        nc.vector.tensor_scalar(
                            out=Bt, in0=A, scalar1=float(R), scalar2=None,
                            op0=AluOpType.is_le,
                        )
                        c2t = u2
                        for gg in range(GB):
                            g = g0 + gg
                            col2 = g * NQ + q
                            col3 = NQ * NG + col2
                            go = gg * QW
                            nc.vector.scalar_tensor_tensor(
                                out=c2t[:, go : go + QW - 1],
                                in0=A[:, go : go + QW - 1],
                                scalar=float(R),
                                in1=Bt[:, go + 1 : go + QW],
                                op0=AluOpType.is_le,
                                op1=AluOpType.mult,
                                accum_out=stats[:, col2 : col2 + 1],
                            )
                            nc.vector.scalar_tensor_tensor(
                                out=S[:, go : go + QW - 2],
                                in0=A[:, go + 2 : go + QW],
                                scalar=float(R),
                                in1=c2t[:, go : go + QW - 2],
                                op0=AluOpType.is_le,
                                op1=AluOpType.mult,
                                accum_out=stats[:, col3 : col3 + 1],
                            )
                    else:
                        m2t = u1
                        ma = m2t[:, :]
                        nc.vector.tensor_tensor(
                            out=gview(ma, QW, 0, 0, QW - 1),
                            in0=gview(aa, QW, 0, 0, QW - 1),
                            in1=gview(aa, QW, 0, 1, QW - 1),
                            op=AluOpType.max,
                        )
                        m3t = u2
                        m3a = m3t[:, :]
                        nc.vector.tensor_tensor(
                            out=gview(m3a, QW, 0, 0, QW - 2),
                            in0=gview(ma, QW, 0, 0, QW - 2),
                            in1=gview(aa, QW, 0, 2, QW - 2),
                            op=AluOpType.max,
                        )
                        for gg in range(GB):
                            g = g0 + gg
                            col2 = g * NQ + q
                            col3 = NQ * NG + col2
                            go = gg * QW
                            nc.scalar.activation(
                                out=S[:, go : go + QW - 1],
                                in_=m2t[:, go : go + QW - 1],
                                func=mybir.ActivationFunctionType.Sign,
                                bias=rbias[:, 0:1],
                                scale=-1.0,
                                accum_out=stats[:, col2 : col2 + 1],
                            )
                            nc.scalar.activation(
                                out=S[:, go : go + QW - 2],
                                in_=m3t[:, go : go + QW - 2],
                                func=mybir.ActivationFunctionType.Sign,
                                bias=rbias[:, 0:1],
                                scale=-1.0,
                                accum_out=stats[:, col3 : col3 + 1],
                            )

            pt = pred.tile([4, 2 * NQ * NG], f32)
            nc.tensor.matmul(pt, selt, stats, start=True, stop=True)
            red = singles.tile([4, 2 * NQ * NG], f32)
            nc.scalar.copy(out=red, in_=pt)
            nc.sync.dma_start(out=out[:, :], in_=red)

    _split_excess_waits(nc)
    return nc


def _get_nc():
    if "nc" not in _CACHE:
        _CACHE["nc"] = _build()
    return _CACHE["nc"]


def _get_runner():
    """Cached jitted 8-core executor: xpad [128, PADLEN] f32 -> [NCORES, 4, 64]."""
    if "fn" in _CACHE:
        return _CACHE["fn"]
    import jax
    import numpy as _np
    from jax.sharding import Mesh, PartitionSpec
    from jax.experimental.shard_map import shard_map
    import concourse.mybir as mybir
    from concourse.bass2jax import (
        _bass_exec_p,
        install_neuronx_cc_hook,
        partition_id_tensor,
    )

    nc = _get_nc()
    install_neuronx_cc_hook()

    in_names, out_names, out_avals, zero_outs = [], [], [], []
    partition_name = nc.partition_id_tensor.name if nc.partition_id_tensor else None
    for alloc in nc.m.functions[0].allocations:
        if not isinstance(alloc, mybir.MemoryLocationSet):
            continue
        name = alloc.memorylocations[0].name
        if alloc.kind == "ExternalInput":
            if name != partition_name:
                in_names.append(name)
        elif alloc.kind == "ExternalOutput":
            shape = tuple(alloc.tensor_shape)
            dtype = mybir.dt.np(alloc.dtype)
            out_names.append(name)
            out_avals.append(jax.core.ShapedArray(shape, dtype))
            zero_outs.append(_np.zeros(shape, dtype))
    n_params = len(in_names)
    n_outs = len(out_avals)
    all_in_names = list(in_names) + list(out_names) + (
        [partition_name] if partition_name else []
    )

    def _body(*args):
        operands = list(args)
        if partition_name is not None:
            operands.append(partition_id_tensor())
        return tuple(
            _bass_exec_p.bind(
                *operands,
                out_avals=tuple(out_avals),
                in_names=tuple(all_in_names),
                out_names=tuple(out_names),
                lowering_input_output_aliases=(),
                sim_require_finite=True,
                sim_require_nnan=True,
                nc=nc,
            )
        )

    devices = jax.devices("axon")[:NCORES]
    mesh = Mesh(np.asarray(devices), ("core",))
    in_specs = (PartitionSpec("core"),) * (n_params + n_outs)
    out_specs = (PartitionSpec("core"),) * n_outs
    fn = jax.jit(
        shard_map(
            _body, mesh=mesh, in_specs=in_specs, out_specs=out_specs, check_rep=False
        ),
        keep_unused=True,
    )
    concat_zeros = [
        np.zeros((NCORES * z.shape[0], *z.shape[1:]), z.dtype) for z in zero_outs
    ]

    def run(xpad):
        out = fn(xpad, *concat_zeros)
        arr = np.asarray(out[0])  # [NCORES*4, 64]
        return arr.reshape(NCORES, 4, 2 * NQ * NG)

    _CACHE["fn"] = run
    return run


def kernel(predictions, targets, _trace=False):
    global LAST_RESULTS

    preds = np.asarray(predictions, dtype=np.float32).reshape(B * C, T)
    targs = np.asarray(targets, dtype=np.float32).reshape(B * C, T)
    xall = np.concatenate([preds, targs], axis=0)  # [128, T]

    mu = xall.mean(axis=1, dtype=np.float64)
    sd = xall.std(axis=1, ddof=1, dtype=np.float64)
    xhat = ((xall - mu[:, None]) / (sd[:, None] + EPS)).astype(np.float32)

    xpad = np.empty((NSIG, PADLEN), dtype=np.float32)
    xpad[:, :T] = xhat
    xpad[:, T:] = 50000.0 + 100.0 * np.arange(PADLEN - T, dtype=np.float32)

    run = _get_runner()
    res = run(np.ascontiguousarray(xpad))
    LAST_RESULTS = res

    # Host spurious-c2 correction (exact fp16 mirror of device math)
    S1 = (xhat[:, 0:N] - xhat[:, N : N + 1]).astype(np.float16)
    S2 = (xhat[:, 1 : N + 1] - xhat[:, N + 1 : N + 2]).astype(np.float16)
    spur2 = (
        (np.abs(S1).astype(np.float32) <= np.float32(R))
        & (np.abs(S2).astype(np.float32) <= np.float32(R))
    ).sum(axis=1)  # [128]

    ents = np.zeros(NSIG, dtype=np.float64)
    for c in range(NCORES):
        o = res[c].astype(np.float64)  # [4, 64]
        for sl in range(S_PER_CORE):
            g, sig = sl // 4, sl % 4
            cnt2 = 0.0
            cnt3 = 0.0
            for q in range(NQ):
                a2 = o[sig, g * NQ + q]
                a3 = o[sig, NQ * NG + g * NQ + q]
                if _is_dve_form(q):
                    cnt2 += a2
                    cnt3 += a3
                else:
                    cnt2 += (a2 + 32.0 * (QW - 1)) / 2.0
                    cnt3 += (a3 + 32.0 * (QW - 2)) / 2.0
            sg = S_PER_CORE * c + sl
            m = 2.0 * (cnt2 - spur2[sg]) + N
            m1 = 2.0 * cnt3 + N
            ratio = m1 / max(m, 1.0)
            ent = -np.log(max(ratio, 1e-30)) if (m > 0 and m1 > 0) else 0.0
            ents[sg] = ent

    ep = ents[: B * C].reshape(B, C)
    et = ents[B * C :].reshape(B, C)
    return np.array(np.mean((ep - et) ** 2), dtype=np.float32)
